# revision 39
# baseline (speedup 1.0000x reference)
"""CLUB loss kernel for Trainium2, 8-core data-parallel SPMD (v5).

Math: with flat_x (N,D) [from x (B,D,H,W) -> (B*H*W, D)], v = exp(-p_logvar),
  loss = (-0.5/N) * [ A - 2B - dot(m2, V) + 2*dot(m1, W) ]
where
  A  = sum_{i,d} x^2 v          B  = sum_{i,d} x mu v
  V_d = sum_i v                 W_d = sum_i mu v
  m1 = S1/N, m2 = S2/N,  S1_d = sum_i x,  S2_d = sum_i x^2
All terms are per-core-local partial sums; the tiny (~KB) cross-core
reduction and final dot products happen on host in float64.

Layout: d-major (partition = d); mu/lv transposed on PE (identity matmuls
into PSUM).  All reductions ride accum_out.  Engine split: ACT = exp +
x^2 + half the S1 copies; DVE = w = muT*v, a = x2*v, b = w*x, other S1s.

All input DMA rides ONE gpsimd (SWDGE) queue in a hand-interleaved order
(issue-time ~0.85us/piece stays ahead of the ~1.5-3us wire time), so
arrival order is exactly the order compute needs: the first (b0,h0) group
is computed at quarter-unit granularity right behind the first two 512K
pieces, x pieces drop in where sq/S1/a/b first need them, and the last
b1 slabs are split along D so the final units' exp/w/a/b chains decouple
dc-pair by dc-pair.  The tail after the last 512K x piece is one short
sq->a->b chain.
"""

import sys

import numpy as np

for _p in ("/opt/trn_rl_repo",):
    if _p not in sys.path:
        sys.path.append(_p)

B, D, H, W = 16, 512, 32, 32
HW = H * W
N = B * HW
NCORES = 8
BLKB = B // NCORES          # b-blocks per core (2)
ROWS = N // NCORES          # rows per core (2048)
NDC = D // 128              # d chunks (4)
NU = BLKB * NDC             # full units per core (8)
HHW = HW // 2               # i-extent of a half-unit (512)
QHW = HW // 4               # i-extent of a quarter-unit (256)

_prog_cache = {}


def build_program():
    import concourse.bacc as bacc
    import concourse.tile as tile
    from concourse import mybir

    f32 = mybir.dt.float32
    AF = mybir.ActivationFunctionType
    OP = mybir.AluOpType

    nc = bacc.Bacc(
        "TRN2",
        target_bir_lowering=False,
        debug=False,
        enable_asserts=False,
        num_devices=NCORES,
    )

    x_d = nc.dram_tensor("x_s", (BLKB, D, HW), f32, kind="ExternalInput").ap()
    mu_d = nc.dram_tensor("mu_s", (ROWS, D), f32, kind="ExternalInput").ap()
    lv_d = nc.dram_tensor("lv_s", (ROWS, D), f32, kind="ExternalInput").ap()
    id_d = nc.dram_tensor("ident", (128, 128), f32, kind="ExternalInput").ap()

    # acc columns (unit u = b*NDC+dc):
    #   V: half (b,h,dc) -> 2u+h in [0,16); quarter (b0,h0,dc,q) -> 16+2dc+q
    #   W: same map + 24                     -> [24,48)
    #   A: 48+u   B: 56+u   S1: 64+u   S2: 72+u
    #   b1 half-gran a/b: a(b1,dc,h) -> 80+4dc+2h, b -> 81+4dc+2h
    # For u<4 (b0) the half-col 2u+0 is unwritten (quarters replace it);
    # for u>=4 the full A/B cols 48+u / 56+u are unwritten (halves used).
    o_misc = nc.dram_tensor("o_misc", (128, 96), f32, kind="ExternalOutput").ap()

    with tile.TileContext(nc) as tc:
        with (
            tc.tile_pool(name="const", bufs=1) as constp,
            tc.tile_pool(name="slab", bufs=4) as slp,
            tc.tile_pool(name="xpool", bufs=2) as xp,
            tc.tile_pool(name="vw", bufs=5) as vwp,
            tc.tile_pool(name="x2p", bufs=5) as x2p,
            tc.tile_pool(name="scr", bufs=2) as scrp,
            tc.tile_pool(name="accum", bufs=1) as accp,
            tc.tile_pool(name="psum", bufs=4, space="PSUM") as pp,
        ):
            ident = constp.tile([128, 128], f32)
            acc = accp.tile([128, 96], f32, tag="acc", name="acc")

            lv_slabs = {}
            mu_slabs = {}
            xq = {}

            # mu/lv slab s covers rows [512s, 512(s+1)) = the (b,h) half
            # with s = 2b+h, stored [128, 4(g), 512(d)] (2 KiB lines).
            # Pieces select i-groups [g0,g1) x d-cols [c0,c1).
            def load_slab(dram, store, s, tag, g0=0, g1=4, c0=0, c1=NDC,
                          eng=None):
                t_ = store.get(s)
                if t_ is None:
                    t_ = slp.tile([128, 4 * D], f32, tag=tag, name=tag)
                    store[s] = t_
                rows = dram[
                    512 * s + 128 * g0 : 512 * s + 128 * g1,
                    128 * c0 : 128 * c1,
                ].rearrange("(g p) f -> p g f", p=128)
                dst = t_[:].rearrange("p (g d) -> p g d", g=4)[
                    :, g0:g1, 128 * c0 : 128 * c1
                ]
                (eng or nc.gpsimd).dma_start(dst, rows)

            # x block b: [128, 4(c), 1024(hw)]; piece = d-chunks [c0,c1)
            def load_x(b, c0, c1):
                t_ = xq.get(b)
                if t_ is None:
                    t_ = xp.tile([128, 4 * HW], f32, tag="x", name="x")
                    xq[b] = t_
                rows = x_d[b, 128 * c0 : 128 * c1, :]
                nc.gpsimd.dma_start(
                    t_[:, c0 * HW : c1 * HW],
                    rows.rearrange("(c p) f -> p c f", p=128),
                )

            # ---- single hand-ordered DMA stream (gpsimd SWDGE queue) ----
            nc.sync.dma_start(ident[:], id_d[:])
            load_slab(mu_d, mu_slabs, 0, "mu_sl", g0=0, g1=2)   # 512K
            load_slab(lv_d, lv_slabs, 0, "lv_sl", g0=0, g1=2)   # 512K
            load_slab(mu_d, mu_slabs, 0, "mu_sl", g0=2, g1=4)   # 512K
            load_slab(lv_d, lv_slabs, 0, "lv_sl", g0=2, g1=4)   # 512K
            load_x(0, 0, 1)                                     # 512K
            load_slab(lv_d, lv_slabs, 1, "lv_sl")               # 1M
            load_slab(mu_d, mu_slabs, 1, "mu_sl")               # 1M
            load_x(0, 1, 3)                                     # 1M
            load_slab(lv_d, lv_slabs, 2, "lv_sl")               # 1M
            load_slab(mu_d, mu_slabs, 2, "mu_sl")               # 1M
            load_x(0, 3, 4)                                     # 512K
            load_x(1, 0, 2)                                     # 1M
            load_x(1, 2, 3)                                     # 512K
            load_slab(lv_d, lv_slabs, 3, "lv_sl", c0=0, c1=2)   # 512K dc01
            load_slab(mu_d, mu_slabs, 3, "mu_sl", c0=0, c1=2)   # 512K dc01
            load_x(1, 3, 4)                                     # 512K
            load_slab(lv_d, lv_slabs, 3, "lv_sl", c0=2, c1=3)   # 256K dc2
            load_slab(mu_d, mu_slabs, 3, "mu_sl", c0=2, c1=3)   # 256K dc2
            load_slab(lv_d, lv_slabs, 3, "lv_sl", c0=3, c1=4)   # 256K dc3
            load_slab(mu_d, mu_slabs, 3, "mu_sl", c0=3, c1=4)   # 256K dc3 (last)

            v_u = {}
            w_u = {}
            x2_u = {}

            def transpose_piece(store, b, h, dc, g0, g1, tag):
                # -> PSUM tile [128, 128*(g1-g0)] = i-cols of the (b,h,dc)
                # unit-half for i-groups [g0,g1)
                s = 2 * b + h
                t_ = pp.tile([128, 128 * (g1 - g0)], f32, tag=tag, name=tag)
                for g in range(g0, g1):
                    nc.tensor.matmul(
                        t_[:, 128 * (g - g0) : 128 * (g - g0 + 1)],
                        store[s][:, D * g + 128 * dc : D * g + 128 * dc + 128],
                        ident[:],
                        is_transpose=True,
                        start=(g == g0),
                        stop=(g == g1 - 1),
                    )
                return t_

            def expw(b, h, dc, g0, g1, vcol):
                # transpose lv+mu piece, exp -> v, w = muT*v, accumulate
                u = b * NDC + dc
                muT = transpose_piece(mu_slabs, b, h, dc, g0, g1, "muT")
                lvT = transpose_piece(lv_slabs, b, h, dc, g0, g1, "lvT")
                o0 = HHW * h + QHW * g0 // 2
                o1 = o0 + QHW * (g1 - g0) // 2
                nc.scalar.activation(
                    v_u[u][:, o0:o1], lvT[:], AF.Exp, scale=-1.0,
                    accum_out=acc[:, vcol : vcol + 1],
                )
                nc.vector.scalar_tensor_tensor(
                    out=w_u[u][:, o0:o1], in0=muT[:], scalar=1.0,
                    in1=v_u[u][:, o0:o1],
                    op0=OP.mult, op1=OP.mult,
                    accum_out=acc[:, 24 + vcol : 24 + vcol + 1],
                )

            def s1pass(b, dc):
                u = b * NDC + dc
                xs = xq[b][:, HW * dc : HW * (dc + 1)]
                s1scr = scrp.tile([128, HW], f32, tag="s1scr", name="s1scr")
                if (b, dc) not in ((0, 0), (0, 1)):
                    nc.scalar.activation(
                        s1scr[:], xs, AF.Copy,
                        accum_out=acc[:, 64 + u : 64 + u + 1],
                    )
                else:
                    nc.vector.tensor_scalar(
                        s1scr[:], xs, 1.0, 0.0, OP.mult, OP.add,
                        accum_out=acc[:, 64 + u : 64 + u + 1],
                    )

            def xpass(b, dc, s1_last=False):
                # sq (ACT, x2 + S2 accum); S1 via ACT copy / DVE
                # tensor_scalar per s1pass
                u = b * NDC + dc
                xs = xq[b][:, HW * dc : HW * (dc + 1)]
                x2_u[u] = x2p.tile([128, HW], f32, tag="x2", name="x2")
                nc.scalar.activation(
                    x2_u[u][:], xs, AF.Square,
                    accum_out=acc[:, 72 + u : 72 + u + 1],
                )
                if not s1_last:
                    s1pass(b, dc)

            def abpass(b, dc, h=None):
                # h=None: full unit; h=0/1: one half (gets its own acc col
                # pair in the spare region [16,24) shifted by +32/+40)
                u = b * NDC + dc
                if h is None:
                    sl = slice(0, HW)
                    ca = 48 + u
                    cb = 56 + u
                else:
                    sl = slice(HHW * h, HHW * (h + 1))
                    ca = 80 + 4 * dc + 2 * h
                    cb = ca + 1
                xs = xq[b][:, HW * dc : HW * dc + HW][:, sl]
                a_scr = scrp.tile([128, sl.stop - sl.start], f32,
                                  tag="ascr", name="ascr")
                nc.vector.scalar_tensor_tensor(
                    out=a_scr[:], in0=x2_u[u][:, sl], scalar=1.0,
                    in1=v_u[u][:, sl],
                    op0=OP.mult, op1=OP.mult,
                    accum_out=acc[:, ca : ca + 1],
                )
                b_scr = scrp.tile([128, sl.stop - sl.start], f32,
                                  tag="bscr", name="bscr")
                nc.vector.scalar_tensor_tensor(
                    out=b_scr[:], in0=w_u[u][:, sl], scalar=1.0, in1=xs,
                    op0=OP.mult, op1=OP.mult,
                    accum_out=acc[:, cb : cb + 1],
                )

            for u in range(NU):
                v_u[u] = vwp.tile([128, HW], f32, tag="v", name="v")
                w_u[u] = vwp.tile([128, HW], f32, tag="w", name="w")

            # ---- compute, emitted in stream-readiness order --------------
            # (b0,h0): quarter granularity right behind the first pieces
            for g0 in (0, 2):
                for dc in range(NDC):
                    expw(0, 0, dc, g0, g0 + 2, 16 + 2 * dc + g0 // 2)
            for dc in range(NDC):        # x0c0 .. x0c3 trickle in
                xpass(0, dc)
            for dc in range(NDC):        # (b0,h1) halves
                expw(0, 1, dc, 0, 4, 2 * dc + 1)
            for dc in range(NDC):        # a/b for b0
                abpass(0, dc)
            # --- b1, interleaved with the dc-split stream tail ---------
            for dc in range(NDC):        # (b1,h0) halves [after s=2 slabs]
                expw(1, 0, dc, 0, 4, 2 * (4 + dc))
            xpass(1, 0)                  # x1c0,c1 land as 1M piece
            xpass(1, 1)
            abpass(1, 0, h=0)            # h0 a/b: v/w h0 + x1 ready
            abpass(1, 1, h=0)
            xpass(1, 2)                  # x1c2 piece
            abpass(1, 2, h=0)
            expw(1, 1, 0, 0, 4, 2 * 4 + 1)      # lv/mu11 dc01 piece
            expw(1, 1, 1, 0, 4, 2 * 5 + 1)
            abpass(1, 0, h=1)
            abpass(1, 1, h=1)
            xpass(1, 3, s1_last=True)    # last x piece (S1 copy deferred)
            abpass(1, 3, h=0)
            expw(1, 1, 2, 0, 4, 2 * 6 + 1)      # lv/mu11 dc2 piece
            abpass(1, 2, h=1)
            expw(1, 1, 3, 0, 4, 2 * 7 + 1)      # lv/mu11 dc3 piece (last)
            abpass(1, 3, h=1)
            s1pass(1, 3)                 # off the critical tail

            nc.sync.dma_start(o_misc[:], acc[:])

    nc.compile()
    return nc


def get_program():
    if "nc" not in _prog_cache:
        _prog_cache["nc"] = build_program()
    return _prog_cache["nc"]


def make_in_maps(x, p_mu, p_logvar):
    x = np.ascontiguousarray(np.asarray(x, dtype=np.float32)).reshape(B, D, HW)
    p_mu = np.ascontiguousarray(np.asarray(p_mu, dtype=np.float32))
    p_logvar = np.ascontiguousarray(np.asarray(p_logvar, dtype=np.float32))
    in_maps = []
    for c in range(NCORES):
        in_maps.append(
            {
                "x_s": np.ascontiguousarray(x[BLKB * c : BLKB * (c + 1)]),
                "mu_s": np.ascontiguousarray(p_mu[ROWS * c : ROWS * (c + 1)]),
                "lv_s": np.ascontiguousarray(p_logvar[ROWS * c : ROWS * (c + 1)]),
                "ident": np.eye(128, dtype=np.float32),
            }
        )
    return in_maps


def finish_host(results):
    """Combine per-core partials (float64) into the scalar loss."""
    Vv = np.zeros(D)
    Ww = np.zeros(D)
    S1 = np.zeros(D)
    S2 = np.zeros(D)
    A = 0.0
    Bb = 0.0
    for r in results:
        misc = r["o_misc"].astype(np.float64)
        for u in range(NU):
            b, dc = divmod(u, NDC)
            dsl = slice(128 * dc, 128 * (dc + 1))
            cols = [2 * u + 1]
            if b == 0:
                cols += [16 + 2 * dc, 16 + 2 * dc + 1]
            else:
                cols += [2 * u]
            for c in cols:
                Vv[dsl] += misc[:, c]
                Ww[dsl] += misc[:, 24 + c]
            if b == 0:
                A += float(misc[:, 48 + u].sum())
                Bb += float(misc[:, 56 + u].sum())
            else:
                for h in range(2):
                    A += float(misc[:, 80 + 4 * dc + 2 * h].sum())
                    Bb += float(misc[:, 81 + 4 * dc + 2 * h].sum())
            S1[dsl] += misc[:, 64 + u]
            S2[dsl] += misc[:, 72 + u]
    m1 = S1 / N
    m2 = S2 / N
    S = A - 2.0 * Bb - float(np.dot(m2, Vv)) + 2.0 * float(np.dot(m1, Ww))
    return np.float32(-0.5 / N * S)


def run_on_device(x, p_mu, p_logvar, trace=False, **kw):
    from concourse import bass_utils

    nc = get_program()
    in_maps = make_in_maps(x, p_mu, p_logvar)
    return bass_utils.run_bass_kernel_spmd(
        nc, in_maps, list(range(NCORES)), trace=trace, **kw
    )


def kernel(x, p_mu, p_logvar):
    res = run_on_device(x, p_mu, p_logvar)
    return finish_host(res.results)


# revision 41
# speedup vs baseline: 1.0488x; 1.0488x over previous
"""CLUB loss kernel for Trainium2, 8-core data-parallel SPMD.

Math: with flat_x (N,D) [from x (B,D,H,W) -> (B*H*W, D)], v = exp(-p_logvar),
  loss = (-0.5/N) * [ A - 2B - dot(m2, V) + 2*dot(m1, W) ]
where
  A  = sum_{i,d} x^2 v          B  = sum_{i,d} x mu v
  V_d = sum_i v                 W_d = sum_i mu v
  m1 = S1/N, m2 = S2/N,  S1_d = sum_i x,  S2_d = sum_i x^2
All terms are per-core-local partial sums; the tiny (~KB) cross-core
reduction and final dot products happen on host in float64.

Layout: d-major (partition = d); mu/lv transposed on PE (identity matmuls
into PSUM).  All reductions ride accum_out on ops that touch the data
anyway (6 elementwise passes total — the minimum for these 6 reductions).
Engine split, tuned so DVE (the critical engine at ~35us of 1x STT work)
carries nothing extra at the end of the stream: ACT = exp (+V), x^2
(+S2), S1-copies for late-arriving units; DVE = w = muT*v (+W),
a = x2*v (+A), b = w*x (+B), plus S1 tensor_scalars for the two
EARLIEST units only (they fill DVE's early stream-pacing gaps).

All input DMA rides ONE gpsimd (SWDGE) queue in a hand-interleaved order
(issue ~0.85us/piece stays ahead of the wire), so arrival order is
exactly the order compute needs: the first (b0,h0) group computes at
quarter-unit granularity right behind the first two 512K pieces, x
pieces drop in where sq/S1/a/b first need them, and the last b1 slabs
are split along D so the final units' exp/w/a/b chains decouple dc-pair
by dc-pair.  b1's a/b run at half-unit granularity emitted in arrival
order, which keeps the post-stream DVE tail to a few short chains.
"""

import sys

import numpy as np

for _p in ("/opt/trn_rl_repo",):
    if _p not in sys.path:
        sys.path.append(_p)

B, D, H, W = 16, 512, 32, 32
HW = H * W
N = B * HW
NCORES = 8
BLKB = B // NCORES          # b-blocks per core (2)
ROWS = N // NCORES          # rows per core (2048)
NDC = D // 128              # d chunks (4)
NU = BLKB * NDC             # full units per core (8)
HHW = HW // 2               # i-extent of a half-unit (512)
QHW = HW // 4               # i-extent of a quarter-unit (256)

_prog_cache = {}


def build_program():
    import concourse.bacc as bacc
    import concourse.tile as tile
    from concourse import mybir

    f32 = mybir.dt.float32
    AF = mybir.ActivationFunctionType
    OP = mybir.AluOpType

    nc = bacc.Bacc(
        "TRN2",
        target_bir_lowering=False,
        debug=False,
        enable_asserts=False,
        num_devices=NCORES,
    )

    x_d = nc.dram_tensor("x_s", (BLKB, D, HW), f32, kind="ExternalInput").ap()
    mu_d = nc.dram_tensor("mu_s", (ROWS, D), f32, kind="ExternalInput").ap()
    lv_d = nc.dram_tensor("lv_s", (ROWS, D), f32, kind="ExternalInput").ap()
    id_d = nc.dram_tensor("ident", (128, 128), f32, kind="ExternalInput").ap()

    # acc columns (unit u = b*NDC+dc):
    #   V: half (b,h,dc) -> 2u+h in [0,16); quarter (b0,h0,dc,q) -> 16+2dc+q
    #   W: same map + 24                     -> [24,48)
    #   A: 48+u   B: 56+u   S1: 64+u   S2: 72+u
    #   b1 half-gran a/b: a(b1,dc,h) -> 80+4dc+2h, b -> 81+4dc+2h
    # For u<4 (b0) the half-col 2u+0 is unwritten (quarters replace it);
    # for u>=4 the full A/B cols 48+u / 56+u are unwritten (halves used).
    o_misc = nc.dram_tensor("o_misc", (128, 96), f32, kind="ExternalOutput").ap()

    with tile.TileContext(nc) as tc:
        with (
            tc.tile_pool(name="const", bufs=1) as constp,
            tc.tile_pool(name="slab", bufs=4) as slp,
            tc.tile_pool(name="xpool", bufs=2) as xp,
            tc.tile_pool(name="vw", bufs=5) as vwp,
            tc.tile_pool(name="x2p", bufs=5) as x2p,
            tc.tile_pool(name="scr", bufs=2) as scrp,
            tc.tile_pool(name="accum", bufs=1) as accp,
            tc.tile_pool(name="psum", bufs=4, space="PSUM") as pp,
        ):
            ident = constp.tile([128, 128], f32)
            acc = accp.tile([128, 96], f32, tag="acc", name="acc")

            lv_slabs = {}
            mu_slabs = {}
            xq = {}

            # mu/lv slab s covers rows [512s, 512(s+1)) = the (b,h) half
            # with s = 2b+h, stored [128, 4(g), 512(d)] (2 KiB lines).
            # Pieces select i-groups [g0,g1) x d-cols [c0,c1).
            def load_slab(dram, store, s, tag, g0=0, g1=4, c0=0, c1=NDC,
                          eng=None):
                t_ = store.get(s)
                if t_ is None:
                    t_ = slp.tile([128, 4 * D], f32, tag=tag, name=tag)
                    store[s] = t_
                rows = dram[
                    512 * s + 128 * g0 : 512 * s + 128 * g1,
                    128 * c0 : 128 * c1,
                ].rearrange("(g p) f -> p g f", p=128)
                dst = t_[:].rearrange("p (g d) -> p g d", g=4)[
                    :, g0:g1, 128 * c0 : 128 * c1
                ]
                (eng or nc.gpsimd).dma_start(dst, rows)

            # x block b: [128, 4(c), 1024(hw)]; piece = d-chunks [c0,c1)
            def load_x(b, c0, c1):
                t_ = xq.get(b)
                if t_ is None:
                    t_ = xp.tile([128, 4 * HW], f32, tag="x", name="x")
                    xq[b] = t_
                rows = x_d[b, 128 * c0 : 128 * c1, :]
                nc.gpsimd.dma_start(
                    t_[:, c0 * HW : c1 * HW],
                    rows.rearrange("(c p) f -> p c f", p=128),
                )

            # ---- single hand-ordered DMA stream (gpsimd SWDGE queue) ----
            nc.sync.dma_start(ident[:], id_d[:])
            load_slab(mu_d, mu_slabs, 0, "mu_sl", g0=0, g1=2)   # 512K
            load_slab(lv_d, lv_slabs, 0, "lv_sl", g0=0, g1=2)   # 512K
            load_slab(mu_d, mu_slabs, 0, "mu_sl", g0=2, g1=4)   # 512K
            load_slab(lv_d, lv_slabs, 0, "lv_sl", g0=2, g1=4)   # 512K
            load_x(0, 0, 1)                                     # 512K
            load_slab(lv_d, lv_slabs, 1, "lv_sl")               # 1M
            load_slab(mu_d, mu_slabs, 1, "mu_sl")               # 1M
            load_x(0, 1, 3)                                     # 1M
            load_slab(lv_d, lv_slabs, 2, "lv_sl")               # 1M
            load_slab(mu_d, mu_slabs, 2, "mu_sl")               # 1M
            load_x(0, 3, 4)                                     # 512K
            load_x(1, 0, 2)                                     # 1M
            load_x(1, 2, 3)                                     # 512K
            load_slab(lv_d, lv_slabs, 3, "lv_sl", c0=0, c1=2)   # 512K dc01
            load_slab(mu_d, mu_slabs, 3, "mu_sl", c0=0, c1=2)   # 512K dc01
            load_x(1, 3, 4)                                     # 512K
            load_slab(lv_d, lv_slabs, 3, "lv_sl", c0=2, c1=4)   # 512K dc23
            load_slab(mu_d, mu_slabs, 3, "mu_sl", c0=2, c1=4)   # 512K dc23 (last)

            v_u = {}
            w_u = {}
            x2_u = {}

            def transpose_piece(store, b, h, dc, g0, g1, tag):
                # -> PSUM tile [128, 128*(g1-g0)] = i-cols of the (b,h,dc)
                # unit-half for i-groups [g0,g1)
                s = 2 * b + h
                t_ = pp.tile([128, 128 * (g1 - g0)], f32, tag=tag, name=tag)
                for g in range(g0, g1):
                    nc.tensor.matmul(
                        t_[:, 128 * (g - g0) : 128 * (g - g0 + 1)],
                        store[s][:, D * g + 128 * dc : D * g + 128 * dc + 128],
                        ident[:],
                        is_transpose=True,
                        start=(g == g0),
                        stop=(g == g1 - 1),
                    )
                return t_

            def expw(b, h, dc, g0, g1, vcol):
                # transpose lv+mu piece, exp -> v, w = muT*v, accumulate
                u = b * NDC + dc
                muT = transpose_piece(mu_slabs, b, h, dc, g0, g1, "muT")
                lvT = transpose_piece(lv_slabs, b, h, dc, g0, g1, "lvT")
                o0 = HHW * h + QHW * g0 // 2
                o1 = o0 + QHW * (g1 - g0) // 2
                nc.scalar.activation(
                    v_u[u][:, o0:o1], lvT[:], AF.Exp, scale=-1.0,
                    accum_out=acc[:, vcol : vcol + 1],
                )
                nc.vector.scalar_tensor_tensor(
                    out=w_u[u][:, o0:o1], in0=muT[:], scalar=1.0,
                    in1=v_u[u][:, o0:o1],
                    op0=OP.mult, op1=OP.mult,
                    accum_out=acc[:, 24 + vcol : 24 + vcol + 1],
                )

            def s1pass(b, dc):
                u = b * NDC + dc
                xs = xq[b][:, HW * dc : HW * (dc + 1)]
                s1scr = scrp.tile([128, HW], f32, tag="s1scr", name="s1scr")
                if (b, dc) not in ((0, 0), (0, 1)):
                    nc.scalar.activation(
                        s1scr[:], xs, AF.Copy,
                        accum_out=acc[:, 64 + u : 64 + u + 1],
                    )
                else:
                    nc.vector.tensor_scalar(
                        s1scr[:], xs, 1.0, 0.0, OP.mult, OP.add,
                        accum_out=acc[:, 64 + u : 64 + u + 1],
                    )

            def xpass(b, dc, s1_last=False):
                # sq (ACT, x2 + S2 accum); S1 via ACT copy / DVE
                # tensor_scalar per s1pass
                u = b * NDC + dc
                xs = xq[b][:, HW * dc : HW * (dc + 1)]
                x2_u[u] = x2p.tile([128, HW], f32, tag="x2", name="x2")
                nc.scalar.activation(
                    x2_u[u][:], xs, AF.Square,
                    accum_out=acc[:, 72 + u : 72 + u + 1],
                )
                if not s1_last:
                    s1pass(b, dc)

            def abpass(b, dc, h=None):
                # h=None: full unit; h=0/1: one half (gets its own acc col
                # pair in the spare region [16,24) shifted by +32/+40)
                u = b * NDC + dc
                if h is None:
                    sl = slice(0, HW)
                    ca = 48 + u
                    cb = 56 + u
                else:
                    sl = slice(HHW * h, HHW * (h + 1))
                    ca = 80 + 4 * dc + 2 * h
                    cb = ca + 1
                xs = xq[b][:, HW * dc : HW * dc + HW][:, sl]
                a_scr = scrp.tile([128, sl.stop - sl.start], f32,
                                  tag="ascr", name="ascr")
                nc.vector.scalar_tensor_tensor(
                    out=a_scr[:], in0=x2_u[u][:, sl], scalar=1.0,
                    in1=v_u[u][:, sl],
                    op0=OP.mult, op1=OP.mult,
                    accum_out=acc[:, ca : ca + 1],
                )
                b_scr = scrp.tile([128, sl.stop - sl.start], f32,
                                  tag="bscr", name="bscr")
                nc.vector.scalar_tensor_tensor(
                    out=b_scr[:], in0=w_u[u][:, sl], scalar=1.0, in1=xs,
                    op0=OP.mult, op1=OP.mult,
                    accum_out=acc[:, cb : cb + 1],
                )

            for u in range(NU):
                v_u[u] = vwp.tile([128, HW], f32, tag="v", name="v")
                w_u[u] = vwp.tile([128, HW], f32, tag="w", name="w")

            # ---- compute, emitted in stream-readiness order --------------
            # (b0,h0): quarter granularity right behind the first pieces
            for g0 in (0, 2):
                for dc in range(NDC):
                    expw(0, 0, dc, g0, g0 + 2, 16 + 2 * dc + g0 // 2)
            for dc in range(NDC):        # x0c0 .. x0c3 trickle in
                xpass(0, dc)
            for dc in range(NDC):        # (b0,h1) halves
                expw(0, 1, dc, 0, 4, 2 * dc + 1)
            for dc in range(NDC):        # a/b for b0
                abpass(0, dc)
            # --- b1, interleaved with the dc-split stream tail ---------
            for dc in range(NDC):        # (b1,h0) halves [after s=2 slabs]
                expw(1, 0, dc, 0, 4, 2 * (4 + dc))
            xpass(1, 0)                  # x1c0,c1 land as 1M piece
            xpass(1, 1)
            abpass(1, 0, h=0)            # h0 a/b: v/w h0 + x1 ready
            abpass(1, 1, h=0)
            xpass(1, 2)                  # x1c2 piece
            abpass(1, 2, h=0)
            expw(1, 1, 0, 0, 4, 2 * 4 + 1)      # lv/mu11 dc01 piece
            expw(1, 1, 1, 0, 4, 2 * 5 + 1)
            abpass(1, 0, h=1)
            abpass(1, 1, h=1)
            xpass(1, 3, s1_last=True)    # last x piece (S1 copy deferred)
            abpass(1, 3, h=0)
            expw(1, 1, 2, 0, 4, 2 * 6 + 1)      # lv/mu11 dc23 piece (last)
            expw(1, 1, 3, 0, 4, 2 * 7 + 1)
            abpass(1, 2, h=1)
            abpass(1, 3, h=1)
            s1pass(1, 3)                 # off the critical tail

            nc.sync.dma_start(o_misc[:], acc[:])

    nc.compile()
    return nc


def get_program():
    if "nc" not in _prog_cache:
        _prog_cache["nc"] = build_program()
    return _prog_cache["nc"]


def make_in_maps(x, p_mu, p_logvar):
    x = np.ascontiguousarray(np.asarray(x, dtype=np.float32)).reshape(B, D, HW)
    p_mu = np.ascontiguousarray(np.asarray(p_mu, dtype=np.float32))
    p_logvar = np.ascontiguousarray(np.asarray(p_logvar, dtype=np.float32))
    in_maps = []
    for c in range(NCORES):
        in_maps.append(
            {
                "x_s": np.ascontiguousarray(x[BLKB * c : BLKB * (c + 1)]),
                "mu_s": np.ascontiguousarray(p_mu[ROWS * c : ROWS * (c + 1)]),
                "lv_s": np.ascontiguousarray(p_logvar[ROWS * c : ROWS * (c + 1)]),
                "ident": np.eye(128, dtype=np.float32),
            }
        )
    return in_maps


def finish_host(results):
    """Combine per-core partials (float64) into the scalar loss."""
    Vv = np.zeros(D)
    Ww = np.zeros(D)
    S1 = np.zeros(D)
    S2 = np.zeros(D)
    A = 0.0
    Bb = 0.0
    for r in results:
        misc = r["o_misc"].astype(np.float64)
        for u in range(NU):
            b, dc = divmod(u, NDC)
            dsl = slice(128 * dc, 128 * (dc + 1))
            cols = [2 * u + 1]
            if b == 0:
                cols += [16 + 2 * dc, 16 + 2 * dc + 1]
            else:
                cols += [2 * u]
            for c in cols:
                Vv[dsl] += misc[:, c]
                Ww[dsl] += misc[:, 24 + c]
            if b == 0:
                A += float(misc[:, 48 + u].sum())
                Bb += float(misc[:, 56 + u].sum())
            else:
                for h in range(2):
                    A += float(misc[:, 80 + 4 * dc + 2 * h].sum())
                    Bb += float(misc[:, 81 + 4 * dc + 2 * h].sum())
            S1[dsl] += misc[:, 64 + u]
            S2[dsl] += misc[:, 72 + u]
    m1 = S1 / N
    m2 = S2 / N
    S = A - 2.0 * Bb - float(np.dot(m2, Vv)) + 2.0 * float(np.dot(m1, Ww))
    return np.float32(-0.5 / N * S)


def run_on_device(x, p_mu, p_logvar, trace=False, **kw):
    from concourse import bass_utils

    nc = get_program()
    in_maps = make_in_maps(x, p_mu, p_logvar)
    return bass_utils.run_bass_kernel_spmd(
        nc, in_maps, list(range(NCORES)), trace=trace, **kw
    )


def kernel(x, p_mu, p_logvar):
    res = run_on_device(x, p_mu, p_logvar)
    return finish_host(res.results)
